# revision 42
# baseline (speedup 1.0000x reference)
"""LGCN (3-layer edge-weighted graph conv, concat features) on 8 TRN2 NeuronCores.

Strategy (graph-partition sharding per spec hint):
- Nodes sharded across 8 cores (12544 = 98x128 rows each); each core owns the
  edges whose dst falls in its shard.
- Per layer: per-edge feature rows are gathered from a replicated HBM node
  table via dma_gather (int16 indices; src space split into 4 chunks of 25088
  rows to fit int16), messages scaled by edge weight on DVE, and scattered
  into the owned node block via a one-hot matmul accumulated in PSUM
  (dst-major edge ordering makes each 128-node block a PSUM accumulation
  group).
- Between layers the computed node shard is AllGather'd into every core's
  node table (halo exchange degenerates to full replication for this
  locality-free random graph).

Host-side preprocessing (numpy) builds the per-core edge arrays (gather
indices, one-hot keys, weights) and a core-shared static loop structure
(tile counts are maxed across cores so the single SPMD program fits all 8
data sets).
"""

import math
import sys

sys.path.insert(0, "/opt/trn_rl_repo")

import numpy as np
import ml_dtypes

from concourse import bass, bacc, mybir, tile
from concourse.bass import AP
from concourse.bass_utils import run_bass_kernel_spmd

P = 128          # SBUF partitions
BLK = 128        # nodes per dst block (PSUM partition dim)
DP = 128         # padded feature columns (bf16) -> 256B gather rows
CH = 4           # src chunks (int16 gather index range)
SLOTS_PER_BANK = 7   # 7 x 64 f32 = 1792B < 2KB PSUM bank
MAX_GRP_BLOCKS = 56  # blocks per drain group (8 banks x 7)
TB = 8           # tiles per gather batch. >8 needs single_packet=False
                 # (single_packet coalesces each engine's descs into ONE
                 # sdma packet; TB=8 = exactly the 64-desc packet ceiling)
                 # but per-desc packets cost ~14% more DMA time: keep 8.
NQ = 4           # SWDGE queues (each on its own Q7 cpu pair)
WBUFS = 4        # work pool depth (gather pipeline)

BF16 = mybir.dt.bfloat16
F32 = mybir.dt.float32
I16 = mybir.dt.int16

SKIP_COLLECTIVES = False  # hang-bisection switch (test only)


class Cfg:
    def __init__(self, n_nodes, d_feat, n_layers, n_cores):
        self.N = n_nodes
        self.D = d_feat
        self.L = n_layers
        self.NC = n_cores
        self.SHARD = int(math.ceil(math.ceil(n_nodes / n_cores) / BLK)) * BLK
        self.BPC = self.SHARD // BLK                   # blocks per core
        self.NG = int(math.ceil(self.BPC / MAX_GRP_BLOCKS))   # drain groups
        self.GBLK = int(math.ceil(self.BPC / self.NG))        # blocks per group
        self.TBL_ROWS = self.NC * self.SHARD
        # Packed-pair table: row r holds nodes (2r, 2r+1), 64 bf16 each, so
        # rows are 256B with zero pad. The CH gather classes are
        # (paired-chunk, parity): even-parity classes gather 256B rows at
        # even node boundaries, odd-parity classes use a +D-element shifted
        # view of the same table (elem_step stays 256B-aligned).
        self.PROWS = self.TBL_ROWS // 2
        assert self.PROWS % (CH // 2) == 0
        self.PCHUNK_R = self.PROWS // (CH // 2)
        assert self.PCHUNK_R <= 32768, "int16 gather index overflow"
        self.DO = (n_layers + 1) * d_feat              # output cols
        # split-allgather: drain-group slices of every core land contiguously
        # in the table so each per-group collective unlocks a chunk pair.
        self.HALF = self.GBLK * BLK
        self.SPLIT = (
            self.NG * self.GBLK == self.BPC
            and (self.NC * self.HALF) % (2 * self.PCHUNK_R) == 0
        )

    def table_row(self, node):
        """Global node id -> (possibly permuted) replicated-table row."""
        if not self.SPLIT:
            return node
        r = node // self.SHARD
        j = node % self.SHARD
        g = j // self.HALF
        return g * (self.NC * self.HALF) + r * self.HALF + (j % self.HALF)


class Plan:
    """Core-shared static structure: segment tile counts and emission order.

    Within each (g, c) span, tiles are ordered by their index-within-segment
    (j ascending): j=0 tiles are full for every core, high-j tiles are the
    pad-richest. This clusters per-core pad edges at the tail of the gather
    batches so the SWDGE trailing-negative-index trim skips their descriptor
    generation and DMA.
    """

    def __init__(self, cfg, seg_tiles):
        # seg_tiles[g][c][b] : tiles for (group, chunk, block-in-group)
        self.cfg = cfg
        self.seg_tiles = seg_tiles
        self.T_total = int(seg_tiles.sum())
        # tile -> (g, c, b, j) in emission order (g-major, then c, then j, b)
        self.tiles = []
        self.spans = {}   # (g, c) -> (t0, t1)
        t = 0
        for g in range(cfg.NG):
            for c in range(CH):
                t0 = t
                for b in range(self._gblocks(g)):
                    for j in range(int(seg_tiles[g, c, b])):
                        self.tiles.append((g, c, b, j))
                        t += 1
                self.spans[(g, c)] = (t0, t)
        # first/last tile per (g, bank) for start/stop flags
        self.first_of_bank = {}
        self.last_of_bank = {}
        for t, (g, c, b, j) in enumerate(self.tiles):
            key = (g, b // SLOTS_PER_BANK)
            if key not in self.first_of_bank:
                self.first_of_bank[key] = t
            self.last_of_bank[key] = t

    def _gblocks(self, g):
        cfg = self.cfg
        return min(cfg.GBLK, cfg.BPC - g * cfg.GBLK)

    def gblocks(self, g):
        return self._gblocks(g)

    def banks(self, g):
        return int(math.ceil(self._gblocks(g) / SLOTS_PER_BANK))


def _exclusive_cumsum(a):
    out = np.zeros_like(a)
    out[1:] = np.cumsum(a)[:-1]
    return out


def preprocess(x, src, dst, w, cfg):
    """Build per-core input maps and the shared Plan."""
    N, NC, SHARD, BPC, NG, GBLK = cfg.N, cfg.NC, cfg.SHARD, cfg.BPC, cfg.NG, cfg.GBLK
    D = cfg.D

    core = dst // SHARD
    blk = (dst % SHARD) // BLK
    grp = blk // GBLK
    b_in_g = blk - grp * GBLK
    trow = cfg.table_row(src)
    prow = trow >> 1
    pchunk = prow // cfg.PCHUNK_R
    # gather class = (paired chunk, src parity); idx = row within the chunk
    chunk = pchunk * 2 + (trow & 1)
    idxval = prow - pchunk * cfg.PCHUNK_R
    dst_rel = dst % BLK

    nkeys = NG * CH * GBLK
    key = (grp * CH + chunk) * GBLK + b_in_g       # per-core segment key
    counts = np.zeros((NC, nkeys), dtype=np.int64)
    for r in range(NC):
        counts[r] = np.bincount(key[core == r], minlength=nkeys)

    seg_tiles = -(-counts.max(axis=0) // BLK).reshape(NG, CH, GBLK)
    # blocks beyond BPC in the last group must have 0 tiles
    for g in range(NG):
        nb = min(GBLK, BPC - g * GBLK)
        seg_tiles[g, :, nb:] = 0
    # every real block needs >=1 tile so its PSUM slot is written
    for g in range(NG):
        nb = min(GBLK, BPC - g * GBLK)
        empty = seg_tiles[g].sum(axis=0)[:nb] == 0
        seg_tiles[g, 0, :nb][empty] = 1

    plan = Plan(cfg, seg_tiles)
    T = plan.T_total
    E_pad = T * BLK
    # (key, j) -> global tile index, following the plan's emission order
    maxj = int(seg_tiles.max())
    tile_of = np.full((nkeys, maxj), -1, dtype=np.int64)
    for t, (g, c, b, j) in enumerate(plan.tiles):
        tile_of[(g * CH + c) * GBLK + b, j] = t

    iota = np.tile(np.arange(P, dtype=np.float32)[None, :], (P, 1)).astype(
        ml_dtypes.bfloat16
    )

    # packed-pair table: node at permuted row t occupies flat 64-elem slot t
    xp = np.zeros(((cfg.PROWS + 1) * 2, D), dtype=ml_dtypes.bfloat16)
    xp[cfg.table_row(np.arange(N))] = x.astype(ml_dtypes.bfloat16)
    x_tbl = xp.reshape(cfg.PROWS + 1, DP)

    idx16s, dstrels, warrs = [], [], []
    last_real = np.zeros(E_pad, dtype=np.int64)  # 1 + last real slot, max cores
    for r in range(NC):
        sel = core == r
        s_key = key[sel]
        s_idx = idxval[sel]
        s_dst_rel = dst_rel[sel]
        s_w = w[sel]

        # lexsort: segment-major, src-row ascending within each segment so
        # gather descriptors walk ascending HBM addresses (DRAM locality)
        order = np.lexsort((s_idx, s_key))
        sk = s_key[order]
        kcnt = np.bincount(sk, minlength=nkeys)
        kstart = _exclusive_cumsum(kcnt)
        rank = np.arange(len(sk)) - kstart[sk]
        pos = tile_of[sk, rank // BLK] * BLK + rank % BLK

        idx16 = np.zeros(E_pad, dtype=np.int16)
        idx16[pos] = s_idx[order].astype(np.int16)
        dstrel = np.full(E_pad, -1.0, dtype=np.float32)
        dstrel[pos] = s_dst_rel[order].astype(np.float32)
        warr = np.zeros(E_pad, dtype=np.float32)
        warr[pos] = s_w[order]
        idx16s.append(idx16)
        dstrels.append(dstrel)
        warrs.append(warr)
        last_real[pos] = np.maximum(last_real[pos], 1)

    # Per-call uniform trim: cut = 1 + last slot that is real on ANY core.
    # Suffix slots >= cut get idx -1 on every core and the call's
    # num_idxs_reg is the SAME constant, keeping the SWDGE ring bookkeeping
    # (reserved from the register) in sync with the descriptors the Q7
    # actually writes after its trailing-negative trim. The first span is
    # left untrimmed so every work buffer is fully written once before any
    # trimmed batch can expose stale SBUF (NaN safety).
    plan.call_cut = {}
    for (g, c), (t0, t1) in plan.spans.items():
        tt = t0
        while tt < t1:
            nt = min(TB, t1 - tt)
            lo, hi = tt * BLK, (tt + nt) * BLK
            if (g, c) == (0, 0):
                cut = nt * BLK
            else:
                nz = np.nonzero(last_real[lo:hi])[0]
                cut = int(nz[-1]) + 1 if len(nz) else 0
            plan.call_cut[tt] = cut
            for r in range(NC):
                idx16s[r][lo + cut:hi] = -1
            tt += nt

    in_maps = []
    for r in range(NC):
        idx_pack = np.tile(idx16s[r].reshape(-1, 16).T, (8, 1))  # [128, T*8]
        dst_pack = dstrels[r].reshape(T, BLK).T.astype(ml_dtypes.bfloat16)
        w_pack = warrs[r].reshape(T, BLK).T.astype(ml_dtypes.bfloat16)

        xs = np.zeros((SHARD, D), dtype=np.float32)
        lo = r * SHARD
        hi = min(N, lo + SHARD)
        if hi > lo:
            xs[: hi - lo] = x[lo:hi]

        in_maps.append(
            {
                "x_tbl": x_tbl,
                "xs": xs,
                "idx": np.ascontiguousarray(idx_pack),
                "dstv": np.ascontiguousarray(dst_pack),
                "wv": np.ascontiguousarray(w_pack),
                "iota": iota,
            }
        )
    return in_maps, plan


def build(cfg, plan):
    """Build the SPMD Bass program (same instruction stream for all cores)."""
    NC, D, T = cfg.NC, cfg.D, plan.T_total
    nc = bacc.Bacc(
        "TRN2", target_bir_lowering=False, debug=False, num_devices=NC,
        num_swdge_queues=NQ, dynamic_dma_scratch_size=65536,
    )

    x_tbl = nc.dram_tensor("x_tbl", [cfg.PROWS + 1, DP], BF16, kind="ExternalInput")
    xs = nc.dram_tensor("xs", [cfg.SHARD, D], F32, kind="ExternalInput")
    idx_d = nc.dram_tensor("idx", [P, T * 8], I16, kind="ExternalInput")
    dst_d = nc.dram_tensor("dstv", [P, T], BF16, kind="ExternalInput")
    w_d = nc.dram_tensor("wv", [P, T], BF16, kind="ExternalInput")
    iota_d = nc.dram_tensor("iota", [P, P], BF16, kind="ExternalInput")
    out_d = nc.dram_tensor("out", [cfg.SHARD, cfg.DO], F32, kind="ExternalOutput")

    shards = [
        nc.dram_tensor(f"hshard{l}", [cfg.SHARD // 2, DP], BF16)
        for l in range(cfg.L - 1)
    ]
    tbls = [
        nc.dram_tensor(f"htbl{l}", [cfg.PROWS + 1, DP], BF16, addr_space="Shared")
        for l in range(cfg.L - 1)
    ]

    core_ids = list(range(NC))

    with tile.TileContext(nc, num_cores=NC) as tc:
        with tc.tile_pool(name="consts", bufs=1) as consts, \
             tc.tile_pool(name="work", bufs=WBUFS) as work, \
             tc.tile_pool(name="stage", bufs=2) as stage, \
             tc.tile_pool(name="ps", bufs=8, space="PSUM") as ps:

            idx_sb = consts.tile([P, T * 8], I16)
            dst_sb = consts.tile([P, T], BF16)
            w_sb = consts.tile([P, T], BF16)
            iota_sb = consts.tile([P, P], BF16)
            nc.sync.dma_start(idx_sb[:], idx_d[:])
            nc.sync.dma_start(dst_sb[:], dst_d[:])
            nc.sync.dma_start(w_sb[:], w_d[:])
            nc.sync.dma_start(iota_sb[:], iota_d[:])

            # out[:, 0:D] = x shard (bounce through SBUF)
            xb = consts.tile([P, cfg.BPC, D], F32)
            nc.sync.dma_start(
                xb[:],
                AP(xs, 0, [[D, P], [BLK * D, cfg.BPC], [1, D]]),
            )
            nc.sync.dma_start(
                AP(out_d, 0, [[cfg.DO, P], [BLK * cfg.DO, cfg.BPC], [1, D]]),
                xb[:],
            )

            qrr = [0]  # SWDGE queue round-robin counter
            # one shared register for the common full-batch num_idxs: avoids
            # a per-gather MOVE on the pool engine's in-order queue
            full_cut_reg = nc.gpsimd.to_reg(TB * BLK)
            # AllGather triggers are deferred into the NEXT group's batch
            # stream: by then their drain-DMA dependencies have landed, so
            # the trigger's waits no longer stall the in-order pool queue
            # between a group's last gather and the next group's first.
            pending_ag = []
            for l in range(cfg.L):
                src_tbl = x_tbl if l == 0 else tbls[l - 1]
                for g in range(cfg.NG):
                    psum_tiles = []
                    for pt in range(plan.banks(g)):
                        psum_tiles.append(
                            ps.tile([P, SLOTS_PER_BANK * D], F32, space="PSUM",
                                    tag="ps", name=f"ps_{l}_{g}_{pt}")
                        )
                    nbatch = 0
                    for c in range(CH):
                        t0, t1 = plan.spans[(g, c)]
                        tt = t0
                        while tt < t1:
                            nt = min(TB, t1 - tt)
                            mg = work.tile([P, TB, DP], BF16, tag="mg")
                            s_eq = work.tile([P, TB, P], BF16, tag="seq")
                            mw = work.tile([P, TB, D], BF16, tag="mw")

                            pchunk, par = c // 2, c % 2
                            nc.gpsimd.dma_gather(
                                out_ap=mg[:, 0:nt, :],
                                in_ap=AP(
                                    src_tbl,
                                    pchunk * cfg.PCHUNK_R * DP + par * D,
                                    [[DP, cfg.PCHUNK_R], [1, DP]],
                                ),
                                idxs_ap=idx_sb[:, tt * 8:(tt + nt) * 8],
                                num_idxs=nt * BLK,
                                num_idxs_reg=(
                                    full_cut_reg
                                    if plan.call_cut[tt] == TB * BLK
                                    else plan.call_cut[tt]
                                ),
                                elem_size=DP,
                                queue_num=qrr[0] % NQ,
                                single_packet=(TB <= 8),
                            )
                            qrr[0] += 1

                            iota_ap = iota_sb[:]
                            iota_b = AP(
                                iota_ap.tensor, iota_ap.offset,
                                [list(iota_ap.ap[0]), [0, nt], [1, P]],
                            )
                            dslice = dst_sb[:, tt:tt + nt]
                            dst_b = AP(
                                dslice.tensor, dslice.offset,
                                [list(dslice.ap[0]), [1, nt], [0, P]],
                            )
                            nc.vector.tensor_tensor(
                                out=s_eq[:, 0:nt, :], in0=iota_b, in1=dst_b,
                                op=mybir.AluOpType.is_equal,
                            )

                            wslice = w_sb[:, tt:tt + nt]
                            w_b = AP(
                                wslice.tensor, wslice.offset,
                                [list(wslice.ap[0]), [1, nt], [0, D]],
                            )
                            nc.vector.tensor_tensor(
                                out=mw[:, 0:nt, :], in0=mg[:, 0:nt, 0:D], in1=w_b,
                                op=mybir.AluOpType.mult,
                            )

                            for k in range(nt):
                                t = tt + k
                                _, _, b, _ = plan.tiles[t]
                                pt, slot = b // SLOTS_PER_BANK, b % SLOTS_PER_BANK
                                nc.tensor.matmul(
                                    out=psum_tiles[pt][:, slot * D:(slot + 1) * D],
                                    lhsT=s_eq[:, k, :],
                                    rhs=mw[:, k, :],
                                    start=(plan.first_of_bank[(g, pt)] == t),
                                    stop=(plan.last_of_bank[(g, pt)] == t),
                                    skip_group_check=True,
                                )
                            tt += nt
                            nbatch += 1
                            if pending_ag and nbatch >= 4:
                                for fn in pending_ag:
                                    fn()
                                pending_ag = []

                    # drains
                    for pt in range(plan.banks(g)):
                        nb = min(SLOTS_PER_BANK, plan.gblocks(g) - pt * SLOTS_PER_BANK)
                        row0 = (g * cfg.GBLK + pt * SLOTS_PER_BANK) * BLK
                        o_st = stage.tile([P, SLOTS_PER_BANK * D], F32, tag="ost")
                        nc.scalar.copy(o_st[:, 0:nb * D], psum_tiles[pt][:, 0:nb * D])
                        nc.sync.dma_start(
                            AP(out_d, row0 * cfg.DO + (l + 1) * D,
                               [[cfg.DO, P], [BLK * cfg.DO, nb], [1, D]]),
                            AP(o_st.tensor, o_st[:].offset,
                               [list(o_st[:].ap[0]), [D, nb], [1, D]]),
                        )
                        if l < cfg.L - 1:
                            h_st = stage.tile([P, SLOTS_PER_BANK * D], BF16, tag="hst")
                            nc.scalar.copy(
                                h_st[:, 0:nb * D], psum_tiles[pt][:, 0:nb * D]
                            )
                            # packed-pair shard: node n at flat 64-elem slot n
                            nc.sync.dma_start(
                                AP(shards[l], row0 * D,
                                   [[D, P], [BLK * D, nb], [1, D]]),
                                AP(h_st.tensor, h_st[:].offset,
                                   [list(h_st[:].ap[0]), [D, nb], [1, D]]),
                            )

                    # per-group-piece allgather: overlaps the next group's
                    # compute and unlocks the next layer's chunk pair early
                    if l < cfg.L - 1 and cfg.SPLIT and not SKIP_COLLECTIVES:
                        hp = cfg.HALF // 2  # paired rows per group piece

                        def _emit_ag(l=l, g=g, hp=hp):
                            nc.gpsimd.collective_compute(
                                "AllGather",
                                mybir.AluOpType.bypass,
                                replica_groups=[core_ids],
                                ins=[shards[l][g * hp:(g + 1) * hp, :]],
                                outs=[tbls[l][g * cfg.NC * hp:
                                              (g + 1) * cfg.NC * hp, :]],
                            )

                        pending_ag.append(_emit_ag)

                if l < cfg.L - 1 and not cfg.SPLIT and not SKIP_COLLECTIVES:
                    nc.gpsimd.collective_compute(
                        "AllGather",
                        mybir.AluOpType.bypass,
                        replica_groups=[core_ids],
                        ins=[shards[l][:]],
                        outs=[tbls[l][0:cfg.PROWS, :]],
                    )

            for fn in pending_ag:
                fn()

    nc.compile()
    return nc


def _run_hw(nc, in_maps, cfg, trace=False):
    res = run_bass_kernel_spmd(
        nc, in_maps, core_ids=list(range(cfg.NC)), trace=trace
    )
    return res


def gnn_kernel(x, edge_index, edge_weight, edge_type, n_layers=3, trace=False):
    x = np.asarray(x, dtype=np.float32)
    src = np.asarray(edge_index[0], dtype=np.int64)
    dst = np.asarray(edge_index[1], dtype=np.int64)
    w = np.asarray(edge_weight, dtype=np.float32)

    cfg = Cfg(x.shape[0], x.shape[1], n_layers, 8)
    in_maps, plan = preprocess(x, src, dst, w, cfg)
    nc = build(cfg, plan)
    global _LAST_NC, _LAST_INMAPS, _LAST_CFG
    _LAST_NC, _LAST_INMAPS, _LAST_CFG = nc, in_maps, cfg
    res = _run_hw(nc, in_maps, cfg, trace=trace)

    parts = []
    for r in range(cfg.NC):
        lo = r * cfg.SHARD
        rows = min(cfg.N - lo, cfg.SHARD)
        parts.append(res.results[r]["out"][:rows])
    out = np.concatenate(parts, axis=0)
    return out, res


def kernel(x, edge_index, edge_weight, edge_type):
    out, _ = gnn_kernel(x, edge_index, edge_weight, edge_type)
    return out

